# revision 12
# baseline (speedup 1.0000x reference)
"""Causal self-attention (B=4, N=2048, D=1024, H=16) on 8 trn2 NeuronCores.

Sharding: core c -> (batch b = c//2, head-group hg = c%2).  Each core runs
one batch with 8 of the 16 heads: QKV projection restricted to its heads'
columns, flash-style causal attention in transposed-score layout, and a
row-parallel output projection producing a partial [N, D] result.  The host
adds the two head-group partials per batch.

Device-side layout choices (all matmuls are lhsT.T @ rhs, PSUM f32):
  - Host feeds x[b].T so Q^T/K^T come out of stage 1 directly (lhsT = W
    tiles, rhs = X^T) and V comes out in natural [seq, head*hd] layout
    (lhsT = X^T tiles, rhs = Wv).  Zero on-device transposes.
  - Scores are computed transposed (ST[j, i] = K^T_tile.T @ Q^T) so the
    softmax denominator is a sum over PSUM *partitions*, which we get for
    free by appending a ones-column to V in the P@V matmul (M=65: 64 hd
    rows + 1 denominator row).
  - Softmax skips the max-subtraction (scores are O(5) here; exp is safe)
    so no partition-axis max is needed.  Fully-masked j-tiles are skipped
    structurally; diagonal-band tiles get a multiplicative {0,1} bf16 mask
    after exp.
  - K-side bias cancels in softmax (constant per query row) and the V-side
    bias commutes through to a host-folded output bias, so only the Q bias
    is applied on device.
  - Q/K/V/P/attn-out/Wproj are stored bf16 in SBUF (matmul accumulation
    stays f32 in PSUM); the QKV projection itself runs in f32.
"""

import numpy as np
import ml_dtypes

import concourse.bass as bass
import concourse.mybir as mybir
import concourse.tile as tile
from concourse import bacc
from concourse.bass_utils import run_bass_kernel_spmd

AF = mybir.ActivationFunctionType
F32 = mybir.dt.float32
F32R = mybir.dt.float32r
BF16 = mybir.dt.bfloat16

B, N, D = 4, 2048, 1024
H, HD = 16, 64
HG = 8                 # heads per core
C = HG * HD            # 512: per-core head width
NT = N // 128          # 16 seq tiles
KT = D // 128          # 8 contraction tiles of the x->qkv matmul
IC = N // 512          # 4 query chunks
SCALE = HD ** -0.5
N_CORES = 8


def build_nc(reps=1):
    nc = bacc.Bacc("TRN2", target_bir_lowering=False, debug=False,
                   num_devices=N_CORES)

    xT = nc.dram_tensor("xT", [D, N], F32R, kind="ExternalInput")
    wq = nc.dram_tensor("wq", [D, C], F32R, kind="ExternalInput")
    wk = nc.dram_tensor("wk", [D, C], F32R, kind="ExternalInput")
    wv = nc.dram_tensor("wv", [D, C], F32R, kind="ExternalInput")
    bq = nc.dram_tensor("bq", [C, 1], F32, kind="ExternalInput")
    wp = nc.dram_tensor("wp", [C, D], BF16, kind="ExternalInput")
    bp = nc.dram_tensor("bp", [128, D], F32, kind="ExternalInput")
    tm = nc.dram_tensor("tm", [128, 2048], BF16, kind="ExternalInput")
    out = nc.dram_tensor("out", [N, D], F32, kind="ExternalOutput")

    with tile.TileContext(nc) as tc:
        with (
            tc.tile_pool(name="persist", bufs=1) as persist,
            tc.tile_pool(name="xt", bufs=2) as xt_pool,
            tc.tile_pool(name="qt", bufs=2) as qt_pool,
            tc.tile_pool(name="aot", bufs=2) as aot_pool,
            tc.tile_pool(name="pt", bufs=4) as pt_pool,
            tc.tile_pool(name="ostage", bufs=4) as ostage_pool,
            tc.tile_pool(name="small", bufs=4) as small_pool,
            tc.tile_pool(name="ps_sc", bufs=3, space="PSUM") as ps_sc,
            tc.tile_pool(name="ps_pv", bufs=1, space="PSUM") as ps_pv,
            tc.tile_pool(name="ps_rep", bufs=1, space="PSUM") as ps_rep,
        ):
            # ---- persistent SBUF tensors ----
            # DMA priority order matters: the first QT accumulation group
            # needs wq[kt] + x^T[kt, chunk0] + bq, so those are interleaved
            # per-kt up front; attention-only tensors (wp/bp/tm) come last
            wq_sb = persist.tile([128, KT, C], F32R)
            wk_sb = persist.tile([128, KT, C], F32R)
            wv_sb = persist.tile([128, KT, C], F32R)

            def dma_xt_chunk(ic):
                xt_t = xt_pool.tile([128, KT, 512], F32R, name="xt")
                for kt in range(KT):
                    nc.sync.dma_start(
                        out=xt_t[:, kt, :],
                        in_=xT[kt * 128:(kt + 1) * 128,
                               ic * 512:(ic + 1) * 512],
                    )
                return xt_t

            for kt in range(KT):
                nc.sync.dma_start(
                    out=wq_sb[:, kt, :],
                    in_=wq[kt * 128:(kt + 1) * 128, :])
            xt0_t = dma_xt_chunk(0)
            bq_sb = persist.tile([128, C // 128], F32)
            nc.sync.dma_start(out=bq_sb[:, :],
                              in_=bq.rearrange("(t p) o -> p (t o)", p=128))
            for kt in range(KT):
                nc.sync.dma_start(
                    out=wk_sb[:, kt, :],
                    in_=wk[kt * 128:(kt + 1) * 128, :])
            for kt in range(KT):
                nc.sync.dma_start(
                    out=wv_sb[:, kt, :],
                    in_=wv[kt * 128:(kt + 1) * 128, :])
            wp_sb = persist.tile([128, C // 128, D], BF16)
            nc.sync.dma_start(out=wp_sb[:, :, :],
                              in_=wp.rearrange("(c p) d -> p c d", p=128))
            bp_sb = persist.tile([128, D], F32)
            nc.sync.dma_start(out=bp_sb[:, :], in_=bp[:, :])
            tm_sb = persist.tile([128, 2048], BF16)
            nc.sync.dma_start(out=tm_sb[:, :], in_=tm[:, :])
            ones_f = persist.tile([1, HD], F32)
            nc.vector.memset(ones_f[:, :], 1.0)
            ones_sb = persist.tile([1, HD], F32R)
            nc.vector.tensor_copy(ones_sb[:, :], ones_f[:, :])

            kt_sb = persist.tile([128, C // 128, N], BF16)   # K^T, c-major
            v_sb = persist.tile([128, NT, HG, HD + 1], BF16)  # V + ones col
            nc.gpsimd.memset(v_sb[:, :, :, HD:HD + 1], 1.0)

            for _rep, ic in [(r, i) for r in range(reps) for i in range(IC)]:
                # ---------- stage 1 for this n-chunk ----------
                xt_t = xt0_t if (_rep == 0 and ic == 0) else dma_xt_chunk(ic)
                qt_t = qt_pool.tile([128, C // 128, 512], BF16, name="qt")

                # Q^T chunk (with bias), K^T (persistent), V (persistent);
                # two accumulation groups share each [128,1024] psum tile
                groups = []
                for ct in range(C // 128):
                    groups.append((
                        lambda p, ct=ct: [nc.tensor.matmul(
                            p, wq_sb[:, kt, ct * 128:(ct + 1) * 128],
                            xt_t[:, kt, :],
                            start=(kt == 0), stop=(kt == KT - 1))
                            for kt in range(KT)],
                        lambda p, ct=ct: nc.vector.tensor_scalar_add(
                            qt_t[:, ct, :], p, bq_sb[:, ct:ct + 1]),
                    ))
                for ct in range(C // 128):
                    groups.append((
                        lambda p, ct=ct: [nc.tensor.matmul(
                            p, wk_sb[:, kt, ct * 128:(ct + 1) * 128],
                            xt_t[:, kt, :],
                            start=(kt == 0), stop=(kt == KT - 1))
                            for kt in range(KT)],
                        lambda p, ct=ct: nc.vector.tensor_copy(
                            kt_sb[:, ct, ic * 512:(ic + 1) * 512], p),
                    ))
                for ntl in range(4):
                    jt = ic * 4 + ntl
                    groups.append((
                        lambda p, ntl=ntl: [nc.tensor.matmul(
                            p, xt_t[:, kt, ntl * 128:(ntl + 1) * 128],
                            wv_sb[:, kt, :],
                            start=(kt == 0), stop=(kt == KT - 1))
                            for kt in range(KT)],
                        lambda p, jt=jt: nc.vector.tensor_copy(
                            v_sb[:, jt, :, 0:HD],
                            p.rearrange("p (h w) -> p h w", w=HD)),
                    ))
                ps_cur = None
                for gi, (emit_mms, evict) in enumerate(groups):
                    if gi % 2 == 0:
                        ps_cur = ps_sc.tile([128, 1024], F32, name="sc")
                    sl = ps_cur[:, 0:512] if gi % 2 == 0 else ps_cur[:, 512:1024]
                    emit_mms(sl)
                    evict(sl)

                # ---------- attention for query chunk ic ----------
                njt = 4 * ic + 4
                aot_t = aot_pool.tile([128, C // 128, 512], BF16, name="aot")
                npair = njt // 2
                for h in range(HG):
                    hh, cth = h % 2, h // 2
                    pv = ps_pv.tile([HD + 1, 512], F32, name="pv")
                    pts = [None] * npair

                    def sc_stage(pr):
                        # two j-tiles' transposed scores into one 2-bank psum
                        # tile -> a single wide exp on the ACT engine
                        ps = ps_sc.tile([128, 1024], F32, name="sc")
                        for u in range(2):
                            jt = 2 * pr + u
                            nc.tensor.matmul(
                                ps[:, u * 512:(u + 1) * 512],
                                kt_sb[hh * 64:hh * 64 + 64, cth,
                                      jt * 128:(jt + 1) * 128],
                                qt_t[hh * 64:hh * 64 + 64, cth, :],
                                start=True, stop=True,
                            )
                        pt = pt_pool.tile([128, 1024], BF16, name="pt")
                        nc.scalar.activation(pt[:, :], ps[:, :], AF.Exp,
                                             scale=SCALE)
                        t0 = 2 * pr - 4 * ic
                        if t0 >= 0:  # diagonal band: mask both halves at once
                            nc.vector.tensor_mul(
                                pt[:, :], pt[:, :],
                                tm_sb[:, t0 * 512:(t0 + 2) * 512])
                        pts[pr] = pt

                    def pv_stage(pr):
                        for u in range(2):
                            jt = 2 * pr + u
                            nc.tensor.matmul(
                                pv[:, :],
                                v_sb[:, jt, h, :],
                                pts[pr][:, u * 512:(u + 1) * 512],
                                start=(jt == 0), stop=(jt == njt - 1),
                            )
                        pts[pr] = None

                    # software pipeline: scores run 2 pairs ahead of P@V so
                    # the PE never waits on the ACT exp of the current pair
                    for pr in range(npair):
                        sc_stage(pr)
                        if pr >= 2:
                            pv_stage(pr - 2)
                    if npair >= 2:
                        pv_stage(npair - 2)
                    pv_stage(npair - 1)

                    # normalize: recip of denominator row, replicate via
                    # a K=1 outer product, multiply into bf16 attn-out
                    rc = small_pool.tile([1, 512], F32R, name="rc")
                    with nc.allow_low_precision(reason="f32r recip, 1e-4 ok"):
                        nc.vector.reciprocal(rc[:, :], pv[HD:HD + 1, :])
                    rep = ps_rep.tile([HD, 512], F32, name="rep")
                    nc.tensor.matmul(rep[:, :], ones_sb[:, :], rc[:, :],
                                     start=True, stop=True)
                    rep_sb = small_pool.tile([HD, 512], F32, name="repsb")
                    nc.vector.tensor_copy(rep_sb[:, :], rep[:, :])
                    nc.vector.tensor_mul(
                        aot_t[hh * 64:hh * 64 + 64, cth, :],
                        pv[0:HD, :], rep_sb[:, :])

                # ---------- output projection for this n-chunk ----------
                for ntl in range(4):
                    nt = ic * 4 + ntl
                    ps = ps_sc.tile([128, 1024], F32, name="sc")
                    for dc in range(2):
                        for ct in range(C // 128):
                            nc.tensor.matmul(
                                ps[:, dc * 512:(dc + 1) * 512],
                                aot_t[:, ct, ntl * 128:(ntl + 1) * 128],
                                wp_sb[:, ct, dc * 512:(dc + 1) * 512],
                                start=(ct == 0), stop=(ct == C // 128 - 1),
                            )
                    ot = ostage_pool.tile([128, 1024], F32, name="ot")
                    nc.vector.tensor_add(ot[:, :], ps[:, :], bp_sb[:, :])
                    nc.sync.dma_start(
                        out=out[nt * 128:(nt + 1) * 128, :], in_=ot[:, :])

    nc.compile()
    return nc


_NC = None


def _get_nc():
    global _NC
    if _NC is None:
        _NC = build_nc()
    return _NC


def _make_tri_masks():
    pj = np.arange(128)[:, None]
    fi = np.arange(512)[None, :]
    blocks = [(fi >= 128 * t + pj) for t in range(4)]
    return np.concatenate(blocks, axis=1).astype(ml_dtypes.bfloat16)


def _numpy_reference(x, causal_mask, Wqkv, bqkv, Wproj, bproj):
    b, n, d = x.shape
    qkv = x @ Wqkv + bqkv
    qkv = qkv.reshape(b, n, 3, H, HD).transpose(2, 0, 3, 1, 4)
    q, k, v = qkv[0], qkv[1], qkv[2]
    s = np.einsum("bhqd,bhkd->bhqk", q, k) * (HD ** -0.5) + causal_mask
    s = s - s.max(axis=-1, keepdims=True)
    p = np.exp(s)
    p /= p.sum(axis=-1, keepdims=True)
    o = np.einsum("bhqk,bhkd->bhqd", p, v)
    o = o.transpose(0, 2, 1, 3).reshape(b, n, d)
    return (o @ Wproj + bproj).astype(np.float32)


def build_in_maps(inputs):
    x = np.asarray(inputs["x"], dtype=np.float32)
    Wqkv = np.asarray(inputs["Wqkv"], dtype=np.float32)
    bqkv = np.asarray(inputs["bqkv"], dtype=np.float32)
    Wproj = np.asarray(inputs["Wproj"], dtype=np.float32)
    bproj = np.asarray(inputs["bproj"], dtype=np.float32)
    tmask = _make_tri_masks()
    xTs = [np.ascontiguousarray(x[b].T) for b in range(B)]
    in_maps = []
    for c in range(N_CORES):
        b, hg = c // 2, c % 2
        cs = slice(hg * C, (hg + 1) * C)
        wp_rows = Wproj[hg * C:(hg + 1) * C, :]
        bv = bqkv[2 * D + hg * C: 2 * D + (hg + 1) * C]
        bp_row = bv @ wp_rows + (bproj if hg == 0 else 0.0)
        in_maps.append({
            "xT": xTs[b],
            "wq": np.ascontiguousarray(Wqkv[:, cs]),
            "wk": np.ascontiguousarray(Wqkv[:, D + hg * C: D + (hg + 1) * C]),
            "wv": np.ascontiguousarray(Wqkv[:, 2 * D + hg * C: 2 * D + (hg + 1) * C]),
            "bq": np.ascontiguousarray(bqkv[cs].reshape(C, 1)),
            "wp": np.ascontiguousarray(wp_rows.astype(ml_dtypes.bfloat16)),
            "bp": np.ascontiguousarray(
                np.broadcast_to(bp_row.astype(np.float32), (128, D))),
            "tm": tmask,
        })
    return in_maps


def kernel(x, causal_mask, Wqkv, bqkv, Wproj, bproj):
    x = np.asarray(x, dtype=np.float32)
    causal_mask = np.asarray(causal_mask, dtype=np.float32)
    Wqkv = np.asarray(Wqkv, dtype=np.float32)
    bqkv = np.asarray(bqkv, dtype=np.float32)
    Wproj = np.asarray(Wproj, dtype=np.float32)
    bproj = np.asarray(bproj, dtype=np.float32)

    # the device kernel applies causality structurally; verify the provided
    # mask is the standard causal mask and fall back to numpy if it isn't
    expected_mask = np.where(
        np.triu(np.ones((N, N), dtype=bool), k=1),
        np.float32(-1e9), np.float32(0.0))
    if causal_mask.shape != (N, N) or not np.array_equal(
            causal_mask, expected_mask):
        return _numpy_reference(x, causal_mask, Wqkv, bqkv, Wproj, bproj)

    nc = _get_nc()
    in_maps = build_in_maps(
        dict(x=x, Wqkv=Wqkv, bqkv=bqkv, Wproj=Wproj, bproj=bproj))

    res = run_bass_kernel_spmd(nc, in_maps, core_ids=list(range(N_CORES)))
    outs = [r["out"] for r in res.results]
    return np.stack([outs[2 * b] + outs[2 * b + 1] for b in range(B)], axis=0)
